# revision 29
# baseline (speedup 1.0000x reference)
"""Calibrated cross-entropy 2D (histogram binning) — Trainium2 Bass kernel.

Problem: nn_CalibratedCE2d_88493506167215
  predict    [8, 21, 513, 513] f32   (NCHW logits)
  target     [8, 513, 513]     int   (class ids)
  confidence [2105352]         f32
  accuracies [15]              f32
  n_bin      15

  loss = -sum_i w_i * logp_target_i / size
  where w_i = coeff[bin(confidence_i)] if selected else 0,
        coeff_b = acc_b*10 - (1-acc_b)*50 (only coeff>0 bins selected),
        size = number of selected pixels.

Key observation: pixels with w_i == 0 (w is a host-side function of
confidence/accuracies only, as in the baseline) contribute nothing to the
loss numerator or denominator, so only the w>0 pixels (~20% for the staged
inputs) need their log-softmax evaluated.  The host packs the alive pixels
of each image into a [128, C*SC] grid, grouped by target class, and uploads
logits as fp8e4 plus a small fp16 sidecar of each pixel's own-class logit
(xseg), making the target-gather a single fused multiply-reduce.

Sharding: one image per NeuronCore, 8 cores (data-parallel over batch).

Per-core device program (grid [128 partitions, FD=C*SC pixel columns];
x arrives in two large DMA transfers so descriptors stay big):
  exp of the 21 class planes is split across two engines (the Pool engine
  is avoided: it contends with DVE for an SBUF port):
    ACT : exact exp (table) for 7 classes
    DVE : Schraudolph fast-exp for 14 classes (y=a*x+b -> int16, bitcast
          to fp16 == 2^(y/1024-15); sigma tuned for zero mean error)
  A-chain (PE): A[p,f] += I @ em_c  (PSUM f32, per-pixel sum over classes;
          identity built on-device via iota, and the PE clock is pre-warmed
          with scratch matmuls so the chain tail runs at peak rate)
  S2 (DVE): one fused stt( xseg * w ) -> acc  (xseg = gathered x_target)
  S1 is linear in bits(A): S1 = FLOG_S*sum(w*bits(A)) - FLOG_B*sum(w), so
          the device only computes T1 = sum(w*float(int32_bits(A))) (DVE)
  out: both accumulators partition-reduced via a ones-matmul to [1, 2] so
          the result DMA is a single 8-byte descriptor
Host: build w/grids, 8-way combine, loss = -(S2-S1)/size.  Inputs that
overflow the grid capacity run in multiple rounds (never for the staged
shapes).
"""

import numpy as np
import ml_dtypes
from contextlib import ExitStack

N_IMG, C, H, W = 8, 21, 513, 513
PX = H * W                    # 263169 pixels per image
N_TOTAL_BINS = 15
SC_DEFAULT = 24               # grid columns per class (capacity 128*SC pixels)
SC_MAX = 64                   # SBUF/PSUM-safe cap; larger inputs run rounds

FEXP_A = float(1024.0 / np.log(2.0))      # fast-exp scale
FEXP_B = float(15 * 1024 - 58.0)          # fast-exp bias (sigma = -58 tuned)
FLOG_S = float(np.log(2.0) / (1 << 23))   # fast-log scale
FLOG_B = 87.991446613                     # fast-log bias (tuned, ~127*ln2)

_NC_CACHE: dict = {}


def _build_program(SC):
    import concourse.bass as bass
    import concourse.bacc as bacc
    import concourse.tile as tile
    from concourse import mybir

    f32 = mybir.dt.float32
    fp16 = mybir.dt.float16
    fp8 = mybir.dt.float8e4
    i16 = mybir.dt.int16
    i32 = mybir.dt.int32
    Exp = mybir.ActivationFunctionType.Exp
    mult = mybir.AluOpType.mult
    add = mybir.AluOpType.add
    bypass = mybir.AluOpType.bypass

    FD = C * SC               # pixel-grid columns
    CHG = 3                   # classes per chunk (one DMA + one exp op each)
    NCH = C // CHG            # 7 chunks; engine per chunk: ACT,DVE,POOL,...
    ENG = ["ACT", "DVE", "DVE", "DVE", "ACT", "DVE", "SPLIT"]
    PE_ORDER = [1, 0, 2, 3, 4, 5, 6]

    nc = bacc.Bacc(
        "TRN2",
        target_bir_lowering=False,
        debug=False,
        enable_asserts=False,
        num_devices=N_IMG,
    )
    x_d = nc.dram_tensor("x", [128, C * FD], fp8, kind="ExternalInput")
    wx_d = nc.dram_tensor("wx", [128, 2 * FD], fp16, kind="ExternalInput")
    out_d = nc.dram_tensor("out", [1, 2], f32, kind="ExternalOutput")

    with tile.TileContext(nc) as tc, ExitStack() as ctx:
        const_pool = ctx.enter_context(tc.tile_pool(name="const", bufs=1))
        empool = ctx.enter_context(tc.tile_pool(name="ep", bufs=8))
        psum = ctx.enter_context(tc.tile_pool(name="ps", bufs=1, space="PSUM"))

        zb = const_pool.tile([128, 1], f32, tag="zb", name="zb")
        nc.vector.memset(zb[:], 0.0)
        # hoist the Exp ACT table load into the DMA-wait window
        dum = const_pool.tile([128, 1], f32, tag="dum", name="dum")
        nc.scalar.activation(dum[:], zb[:], Exp, bias=zb[:, 0:1])

        xall = const_pool.tile([128, C * FD], fp8, tag="xall", name="xall")
        wx = const_pool.tile([128, 2 * FD], fp16, tag="wx", name="wx")
        w_t = wx[:, 0:FD]
        xseg = wx[:, FD : 2 * FD]
        acc = const_pool.tile([128, 2], f32, tag="acc", name="acc")

        A = psum.tile([128, FD], f32, tag="A", name="A")
        Aw = psum.tile([128, FD], f32, tag="Aw", name="Aw")

        # identity built on-device: iota(col - row) == 0
        iti = const_pool.tile([128, 128], mybir.dt.int16, tag="iti", name="iti")
        nc.gpsimd.iota(iti[:], [[1, 128]], base=0, channel_multiplier=-1)
        idt = const_pool.tile([128, 128], fp16, tag="idt", name="idt")
        nc.vector.tensor_scalar(idt[:], iti[:], 0, None, op0=mybir.AluOpType.is_equal)
        # warm source + PE warm-up chain: ramps the PE clock to full speed
        # before the real A-chain so its tail runs at peak rate
        wsrc = const_pool.tile([128, FD], fp16, tag="wsrc", name="wsrc")
        nc.gpsimd.memset(wsrc[:], 0.0)
        for u in range(8):
            nc.tensor.matmul(Aw[:], idt[:], wsrc[:], start=(u == 0), stop=(u == 7))

        x = x_d.ap()
        CW = CHG * FD
        # two big x transfers (large descriptors amortize the per-descriptor
        # DMA overhead): chunks 0-3, then 4-6
        nc.sync.dma_start(xall[:, 0 : 4 * CW], x[:, 0 : 4 * CW])
        nc.sync.dma_start(xall[:, 4 * CW :], x[:, 4 * CW :])
        nc.sync.dma_start(wx[:], wx_d.ap())

        nslice = (FD + 511) // 512
        emvs = {}
        for k in range(NCH):
            xc = xall[:, k * CW : (k + 1) * CW]
            if ENG[k] == "ACT":
                em = empool.tile([128, CW], fp16, tag=f"emA{k}", name=f"em{k}")
                nc.scalar.activation(em[:], xc, Exp, bias=zb[:, 0:1])
                emvs[k] = em[:]
            elif ENG[k] == "DVE":
                em = empool.tile([128, CW], i16, tag=f"emD{k}", name=f"em{k}")
                nc.vector.tensor_scalar(em[:], xc, FEXP_A, FEXP_B, op0=mult, op1=add)
                emvs[k] = em[:].bitcast(fp16)
            else:  # SPLIT: first class on ACT (exact), rest on DVE fast-exp
                ema = empool.tile([128, FD], fp16, tag=f"emA{k}", name=f"ema{k}")
                nc.scalar.activation(ema[:], xall[:, k * CW : k * CW + FD], Exp,
                                     bias=zb[:, 0:1])
                emb = empool.tile([128, 2 * FD], i16, tag=f"emD{k}", name=f"emb{k}")
                nc.vector.tensor_scalar(
                    emb[:], xall[:, k * CW + FD : (k + 1) * CW],
                    FEXP_A, FEXP_B, op0=mult, op1=add,
                )
                emvs[k] = (ema[:], emb[:].bitcast(fp16))
        for i, k in enumerate(PE_ORDER):
            ev = emvs[k]
            parts = (
                [(ev, j) for j in range(CHG)]
                if not isinstance(ev, tuple)
                else [(ev[0], 0)] + [(ev[1], j) for j in range(CHG - 1)]
            )
            for pj, (emv, j) in enumerate(parts):
                for s in range(nslice):
                    sl = slice(s * 512, min(FD, (s + 1) * 512))
                    nc.tensor.matmul(
                        A[:, sl], idt[:], emv[:, j * FD + sl.start : j * FD + sl.stop],
                        start=(i == 0 and pj == 0),
                        stop=(i == NCH - 1 and pj == len(parts) - 1),
                    )

        # S2: one fused multiply-accumulate over the gathered target logits
        scr = const_pool.tile([128, FD], fp16, tag="scr", name="scr")
        nc.vector.scalar_tensor_tensor(
            scr[:], xseg[:], 0.0, w_t[:], op0=bypass, op1=mult,
            accum_out=acc[:, 1:2],
        )

        # S1 is linear in bits(A):  S1 = FLOG_S * sum(w*bits) - FLOG_B * sum(w)
        # -> device only needs T1 = sum w*float(int32_bits(A)); host finishes.
        scr2 = const_pool.tile([128, FD], f32, tag="scr2", name="scr2")
        nc.vector.scalar_tensor_tensor(
            scr2[:], A[:].bitcast(i32), 0.0, w_t[:], op0=bypass, op1=mult,
            accum_out=acc[:, 0:1],
        )
        # partition-reduce acc via ones-matmul -> [1, 2], then 8-byte out DMA
        ones = const_pool.tile([128, 1], f32, tag="ones", name="ones")
        nc.vector.memset(ones[:], 1.0)
        accr = psum.tile([1, 2], f32, tag="accr", name="accr")
        nc.tensor.matmul(accr[:], ones[:], acc[:], start=True, stop=True)
        outs = const_pool.tile([1, 2], f32, tag="outs", name="outs")
        nc.vector.tensor_scalar(outs[:], accr[:], 0.0, None, op0=add)
        nc.sync.dma_start(out_d.ap(), outs[:])

    nc.compile()
    return nc


def _get_nc(SC=SC_DEFAULT):
    if SC not in _NC_CACHE:
        _NC_CACHE[SC] = _build_program(SC)
    return _NC_CACHE[SC]


def _pixel_weights(conf: np.ndarray, accuracies: np.ndarray, n_bin: int):
    """Per-pixel weights, f32 arithmetic identical to the reference."""
    acc = np.asarray(accuracies, dtype=np.float32)[:n_bin]
    coeff = acc * np.float32(10.0) - (np.float32(1.0) - acc) * np.float32(50.0)
    wtab = np.where(coeff > np.float32(0.0), coeff, np.float32(0.0)).astype(np.float32)
    # table16[k] for k = ceil(conf*15) in 0..15; k=0 (conf==0) -> invalid -> 0
    table16 = np.concatenate([[np.float32(0.0)], wtab]).astype(np.float32)
    t15 = conf * np.float32(N_TOTAL_BINS)          # same f32 product as reference
    k16 = np.ceil(t15).astype(np.int32)
    k16 = np.clip(k16, 0, n_bin)
    wfull = table16[k16]
    valid = (conf > np.float32(0.0)) & (conf <= np.float32(1.0))
    wfull = np.where(valid, wfull, np.float32(0.0)).astype(np.float32)
    return wfull


def _prepare(predict, target, confidence, accuracies, n_bin):
    predict = np.ascontiguousarray(np.asarray(predict, dtype=np.float32))
    target = np.asarray(target)
    conf = np.asarray(confidence, dtype=np.float32)
    accuracies = np.asarray(accuracies, dtype=np.float32)
    n_bin = int(n_bin)
    assert predict.shape == (N_IMG, C, H, W) and n_bin == N_TOTAL_BINS

    wfull = _pixel_weights(conf, accuracies, n_bin)
    size = float(np.count_nonzero(wfull))

    xs = predict.reshape(N_IMG, C, PX)
    tg = target.reshape(N_IMG, PX).astype(np.int32)
    wf = wfull.reshape(N_IMG, PX)

    # capacity needed: max alive-per-class count across cores
    maxcnt = 1
    percore = []
    for n in range(N_IMG):
        alive = np.nonzero(wf[n])[0].astype(np.int32)
        ta = tg[n][alive]
        order = np.argsort(ta, kind="stable")
        ids = alive[order]
        cnts = np.bincount(ta, minlength=C)
        maxcnt = max(maxcnt, int(cnts.max()))
        percore.append((ids, cnts))
    if maxcnt <= 128 * SC_DEFAULT:
        SC = SC_DEFAULT
    else:
        SC = min((maxcnt + 127) // 128 + 2, SC_MAX)
    FD = C * SC
    cap = 128 * SC
    rounds = (maxcnt + cap - 1) // cap

    round_maps = []
    round_wsums = []
    for r in range(rounds):
        in_maps = []
        wsums = []
        for n in range(N_IMG):
            ids, cnts = percore[n]
            pixgrid = np.zeros((128, FD), dtype=np.int64)  # pad -> pixel 0 (w=0)
            wgrid = np.zeros((128, FD), dtype=np.float16)
            off = 0
            for c in range(C):
                ncnt = int(cnts[c])
                seg = ids[off : off + ncnt][r * cap : (r + 1) * cap]
                off += ncnt
                ncnt = len(seg)
                s = np.arange(ncnt)
                p, f = s % 128, c * SC + s // 128
                pixgrid[p, f] = seg
                wgrid[p, f] = wf[n][seg]
            g = xs[n][:, pixgrid.reshape(-1)]            # [C, 128*FD]
            g = g.reshape(C, 128, FD).transpose(1, 0, 2)  # [128, C, FD]
            xg = np.ascontiguousarray(g.reshape(128, C * FD)).astype(
                ml_dtypes.float8_e4m3fn
            )
            # xseg: each grid pixel's own-class logit (exact f32 -> fp16)
            xseg = np.zeros((128, FD), dtype=np.float16)
            for c in range(C):
                sl = slice(c * SC, (c + 1) * SC)
                xseg[:, sl] = g[:, c, sl].astype(np.float16)
            wx = np.ascontiguousarray(np.concatenate([wgrid, xseg], axis=1))
            wsums.append(float(wgrid.astype(np.float64).sum()))
            in_maps.append({"x": xg, "wx": wx})
        round_maps.append(in_maps)
        round_wsums.append(wsums)
    return round_maps, size, SC, round_wsums


def _combine_sum(res_list, wsums) -> float:
    S = 0.0
    for n in range(N_IMG):
        o = np.asarray(res_list[n]["out"], dtype=np.float64)
        S1 = FLOG_S * o[0, 0] - FLOG_B * wsums[n]
        S += o[0, 1] - S1                            # S2 - S1
    return S


def _combine(res_list, size, wsums) -> np.ndarray:
    loss = np.float32(-(_combine_sum(res_list, wsums) / size))
    return np.asarray(loss, dtype=np.float32)


def run_device(in_maps, SC=SC_DEFAULT, trace=False, **kwargs):
    from concourse.bass_utils import run_bass_kernel_spmd

    nc = _get_nc(SC)
    return run_bass_kernel_spmd(
        nc, in_maps, core_ids=list(range(N_IMG)), trace=trace, **kwargs
    )


def kernel(predict, target, confidence, accuracies, n_bin) -> np.ndarray:
    round_maps, size, SC, round_wsums = _prepare(
        predict, target, confidence, accuracies, n_bin
    )
    S = 0.0
    for in_maps, wsums in zip(round_maps, round_wsums):
        res = run_device(in_maps, SC=SC)
        S += _combine_sum(res.results, wsums)
    return np.asarray(np.float32(-(S / size)), dtype=np.float32)


# revision 30
# speedup vs baseline: 1.0856x; 1.0856x over previous
"""Calibrated cross-entropy 2D (histogram binning) — Trainium2 Bass kernel.

Problem: nn_CalibratedCE2d_88493506167215
  predict    [8, 21, 513, 513] f32   (NCHW logits)
  target     [8, 513, 513]     int   (class ids)
  confidence [2105352]         f32
  accuracies [15]              f32
  n_bin      15

  loss = -sum_i w_i * logp_target_i / size
  where w_i = coeff[bin(confidence_i)] if selected else 0,
        coeff_b = acc_b*10 - (1-acc_b)*50 (only coeff>0 bins selected),
        size = number of selected pixels.

Key observation: pixels with w_i == 0 (w is a host-side function of
confidence/accuracies only, as in the baseline) contribute nothing to the
loss numerator or denominator, so only the w>0 pixels (~20% for the staged
inputs) need their log-softmax evaluated.  The host packs the alive pixels
of each image into a [128, C*SC] grid, grouped by target class, and uploads
logits as fp8e4 plus a small fp16 sidecar of each pixel's own-class logit
(xseg), making the target-gather a single fused multiply-reduce.

Sharding: one image per NeuronCore, 8 cores (data-parallel over batch).

Per-core device program (grid [128 partitions, FD=C*SC pixel columns];
x arrives in two large DMA transfers so descriptors stay big):
  exp of the 21 class planes is split across two engines (the Pool engine
  is avoided: it contends with DVE for an SBUF port):
    ACT : exact exp (table) for 7 classes
    DVE : Schraudolph fast-exp for 14 classes (y=a*x+b -> int16, bitcast
          to fp16 == 2^(y/1024-15); sigma tuned for zero mean error)
  A-chain (PE): A[p,f] += I @ em_c  (PSUM f32, per-pixel sum over classes;
          identity built on-device via iota, and the PE clock is pre-warmed
          with scratch matmuls so the chain tail runs at peak rate)
  S2 (DVE): one fused stt( xseg * w ) -> acc  (xseg = gathered x_target)
  S1 is linear in bits(A): S1 = FLOG_S*sum(w*bits(A)) - FLOG_B*sum(w), so
          the device only computes T1 = sum(w*float(int32_bits(A))) (DVE)
  out: both accumulators partition-reduced via a ones-matmul to [1, 2] so
          the result DMA is a single 8-byte descriptor
Host: build w/grids, 8-way combine, loss = -(S2-S1)/size.  Inputs that
overflow the grid capacity run in multiple rounds (never for the staged
shapes).
"""

import numpy as np
import ml_dtypes
from contextlib import ExitStack

N_IMG, C, H, W = 8, 21, 513, 513
PX = H * W                    # 263169 pixels per image
N_TOTAL_BINS = 15
SC_DEFAULT = 24               # grid columns per class (capacity 128*SC pixels)
SC_MAX = 64                   # SBUF/PSUM-safe cap; larger inputs run rounds

FEXP_A = float(1024.0 / np.log(2.0))      # fast-exp scale
FEXP_B = float(15 * 1024 - 58.0)          # fast-exp bias (sigma = -58 tuned)
FLOG_S = float(np.log(2.0) / (1 << 23))   # fast-log scale
FLOG_B = 87.991446613                     # fast-log bias (tuned, ~127*ln2)

_NC_CACHE: dict = {}


def _build_program(SC):
    import concourse.bass as bass
    import concourse.bacc as bacc
    import concourse.tile as tile
    from concourse import mybir

    f32 = mybir.dt.float32
    fp16 = mybir.dt.float16
    fp8 = mybir.dt.float8e4
    i16 = mybir.dt.int16
    i32 = mybir.dt.int32
    Exp = mybir.ActivationFunctionType.Exp
    mult = mybir.AluOpType.mult
    add = mybir.AluOpType.add
    bypass = mybir.AluOpType.bypass

    FD = C * SC               # pixel-grid columns
    CHG = 3                   # classes per chunk (one DMA + one exp op each)
    NCH = C // CHG            # 7 chunks; engine per chunk: ACT,DVE,POOL,...
    ENG = ["ACT", "DVE", "DVE", "DVE", "ACT", "DVE", "SPLIT"]
    PE_ORDER = [1, 0, 2, 3, 4, 5, 6]

    nc = bacc.Bacc(
        "TRN2",
        target_bir_lowering=False,
        debug=False,
        enable_asserts=False,
        num_devices=N_IMG,
    )
    x_d = nc.dram_tensor("x", [128, C * FD], fp8, kind="ExternalInput")
    wx_d = nc.dram_tensor("wx", [128, 2 * FD], fp16, kind="ExternalInput")
    out_d = nc.dram_tensor("out", [1, 2], f32, kind="ExternalOutput")

    with tile.TileContext(nc) as tc, ExitStack() as ctx:
        const_pool = ctx.enter_context(tc.tile_pool(name="const", bufs=1))
        empool = ctx.enter_context(tc.tile_pool(name="ep", bufs=8))
        psum = ctx.enter_context(tc.tile_pool(name="ps", bufs=1, space="PSUM"))

        zb = const_pool.tile([128, 1], f32, tag="zb", name="zb")
        nc.vector.memset(zb[:], 0.0)
        # hoist the Exp ACT table load into the DMA-wait window
        dum = const_pool.tile([128, 1], f32, tag="dum", name="dum")
        nc.scalar.activation(dum[:], zb[:], Exp, bias=zb[:, 0:1])

        xall = const_pool.tile([128, C * FD], fp8, tag="xall", name="xall")
        wx = const_pool.tile([128, 2 * FD], fp16, tag="wx", name="wx")
        w_t = wx[:, 0:FD]
        xseg = wx[:, FD : 2 * FD]
        acc = const_pool.tile([128, 2], f32, tag="acc", name="acc")

        A = psum.tile([128, FD], f32, tag="A", name="A")
        Aw = psum.tile([128, FD], f32, tag="Aw", name="Aw")

        # identity built on-device: iota(col - row) == 0
        iti = const_pool.tile([128, 128], mybir.dt.int16, tag="iti", name="iti")
        nc.gpsimd.iota(iti[:], [[1, 128]], base=0, channel_multiplier=-1)
        idt = const_pool.tile([128, 128], fp16, tag="idt", name="idt")
        nc.vector.tensor_scalar(idt[:], iti[:], 0, None, op0=mybir.AluOpType.is_equal)
        # warm source + PE warm-up chain: ramps the PE clock to full speed
        # before the real A-chain so its tail runs at peak rate
        wsrc = const_pool.tile([128, FD], fp16, tag="wsrc", name="wsrc")
        nc.gpsimd.memset(wsrc[:], 0.0)
        for u in range(9):
            nc.tensor.matmul(Aw[:], idt[:], wsrc[:], start=(u == 0), stop=(u == 8))

        x = x_d.ap()
        CW = CHG * FD
        # two big x transfers (large descriptors amortize the per-descriptor
        # DMA overhead): chunks 0-3, then 4-6
        nc.sync.dma_start(xall[:, 0 : 4 * CW], x[:, 0 : 4 * CW])
        nc.sync.dma_start(xall[:, 4 * CW :], x[:, 4 * CW :])
        nc.sync.dma_start(wx[:], wx_d.ap())

        nslice = (FD + 511) // 512
        emvs = {}
        for k in range(NCH):
            xc = xall[:, k * CW : (k + 1) * CW]
            if ENG[k] == "ACT":
                em = empool.tile([128, CW], fp16, tag=f"emA{k}", name=f"em{k}")
                nc.scalar.activation(em[:], xc, Exp, bias=zb[:, 0:1])
                emvs[k] = em[:]
            elif ENG[k] == "DVE":
                em = empool.tile([128, CW], i16, tag=f"emD{k}", name=f"em{k}")
                nc.vector.tensor_scalar(em[:], xc, FEXP_A, FEXP_B, op0=mult, op1=add)
                emvs[k] = em[:].bitcast(fp16)
            else:  # SPLIT: first class on ACT (exact), rest on DVE fast-exp
                ema = empool.tile([128, FD], fp16, tag=f"emA{k}", name=f"ema{k}")
                nc.scalar.activation(ema[:], xall[:, k * CW : k * CW + FD], Exp,
                                     bias=zb[:, 0:1])
                emb = empool.tile([128, 2 * FD], i16, tag=f"emD{k}", name=f"emb{k}")
                nc.vector.tensor_scalar(
                    emb[:], xall[:, k * CW + FD : (k + 1) * CW],
                    FEXP_A, FEXP_B, op0=mult, op1=add,
                )
                emvs[k] = (ema[:], emb[:].bitcast(fp16))
        for i, k in enumerate(PE_ORDER):
            ev = emvs[k]
            parts = (
                [(ev, j) for j in range(CHG)]
                if not isinstance(ev, tuple)
                else [(ev[0], 0)] + [(ev[1], j) for j in range(CHG - 1)]
            )
            for pj, (emv, j) in enumerate(parts):
                for s in range(nslice):
                    sl = slice(s * 512, min(FD, (s + 1) * 512))
                    nc.tensor.matmul(
                        A[:, sl], idt[:], emv[:, j * FD + sl.start : j * FD + sl.stop],
                        start=(i == 0 and pj == 0),
                        stop=(i == NCH - 1 and pj == len(parts) - 1),
                    )

        # S2: one fused multiply-accumulate over the gathered target logits
        scr = const_pool.tile([128, FD], fp16, tag="scr", name="scr")
        nc.vector.scalar_tensor_tensor(
            scr[:], xseg[:], 0.0, w_t[:], op0=bypass, op1=mult,
            accum_out=acc[:, 1:2],
        )

        # S1 is linear in bits(A):  S1 = FLOG_S * sum(w*bits) - FLOG_B * sum(w)
        # -> device only needs T1 = sum w*float(int32_bits(A)); host finishes.
        scr2 = const_pool.tile([128, FD], f32, tag="scr2", name="scr2")
        nc.vector.scalar_tensor_tensor(
            scr2[:], A[:].bitcast(i32), 0.0, w_t[:], op0=bypass, op1=mult,
            accum_out=acc[:, 0:1],
        )
        # partition-reduce acc via ones-matmul -> [1, 2], then 8-byte out DMA
        ones = const_pool.tile([128, 1], f32, tag="ones", name="ones")
        nc.vector.memset(ones[:], 1.0)
        accr = psum.tile([1, 2], f32, tag="accr", name="accr")
        nc.tensor.matmul(accr[:], ones[:], acc[:], start=True, stop=True)
        outs = const_pool.tile([1, 2], f32, tag="outs", name="outs")
        nc.vector.tensor_scalar(outs[:], accr[:], 0.0, None, op0=add)
        nc.sync.dma_start(out_d.ap(), outs[:])

    nc.compile()
    return nc


def _get_nc(SC=SC_DEFAULT):
    if SC not in _NC_CACHE:
        _NC_CACHE[SC] = _build_program(SC)
    return _NC_CACHE[SC]


def _pixel_weights(conf: np.ndarray, accuracies: np.ndarray, n_bin: int):
    """Per-pixel weights, f32 arithmetic identical to the reference."""
    acc = np.asarray(accuracies, dtype=np.float32)[:n_bin]
    coeff = acc * np.float32(10.0) - (np.float32(1.0) - acc) * np.float32(50.0)
    wtab = np.where(coeff > np.float32(0.0), coeff, np.float32(0.0)).astype(np.float32)
    # table16[k] for k = ceil(conf*15) in 0..15; k=0 (conf==0) -> invalid -> 0
    table16 = np.concatenate([[np.float32(0.0)], wtab]).astype(np.float32)
    t15 = conf * np.float32(N_TOTAL_BINS)          # same f32 product as reference
    k16 = np.ceil(t15).astype(np.int32)
    k16 = np.clip(k16, 0, n_bin)
    wfull = table16[k16]
    valid = (conf > np.float32(0.0)) & (conf <= np.float32(1.0))
    wfull = np.where(valid, wfull, np.float32(0.0)).astype(np.float32)
    return wfull


def _prepare(predict, target, confidence, accuracies, n_bin):
    predict = np.ascontiguousarray(np.asarray(predict, dtype=np.float32))
    target = np.asarray(target)
    conf = np.asarray(confidence, dtype=np.float32)
    accuracies = np.asarray(accuracies, dtype=np.float32)
    n_bin = int(n_bin)
    assert predict.shape == (N_IMG, C, H, W) and n_bin == N_TOTAL_BINS

    wfull = _pixel_weights(conf, accuracies, n_bin)
    size = float(np.count_nonzero(wfull))

    xs = predict.reshape(N_IMG, C, PX)
    tg = target.reshape(N_IMG, PX).astype(np.int32)
    wf = wfull.reshape(N_IMG, PX)

    # capacity needed: max alive-per-class count across cores
    maxcnt = 1
    percore = []
    for n in range(N_IMG):
        alive = np.nonzero(wf[n])[0].astype(np.int32)
        ta = tg[n][alive]
        order = np.argsort(ta, kind="stable")
        ids = alive[order]
        cnts = np.bincount(ta, minlength=C)
        maxcnt = max(maxcnt, int(cnts.max()))
        percore.append((ids, cnts))
    if maxcnt <= 128 * SC_DEFAULT:
        SC = SC_DEFAULT
    else:
        SC = min((maxcnt + 127) // 128 + 2, SC_MAX)
    FD = C * SC
    cap = 128 * SC
    rounds = (maxcnt + cap - 1) // cap

    round_maps = []
    round_wsums = []
    for r in range(rounds):
        in_maps = []
        wsums = []
        for n in range(N_IMG):
            ids, cnts = percore[n]
            pixgrid = np.zeros((128, FD), dtype=np.int64)  # pad -> pixel 0 (w=0)
            wgrid = np.zeros((128, FD), dtype=np.float16)
            off = 0
            for c in range(C):
                ncnt = int(cnts[c])
                seg = ids[off : off + ncnt][r * cap : (r + 1) * cap]
                off += ncnt
                ncnt = len(seg)
                s = np.arange(ncnt)
                p, f = s % 128, c * SC + s // 128
                pixgrid[p, f] = seg
                wgrid[p, f] = wf[n][seg]
            g = xs[n][:, pixgrid.reshape(-1)]            # [C, 128*FD]
            g = g.reshape(C, 128, FD).transpose(1, 0, 2)  # [128, C, FD]
            xg = np.ascontiguousarray(g.reshape(128, C * FD)).astype(
                ml_dtypes.float8_e4m3fn
            )
            # xseg: each grid pixel's own-class logit (exact f32 -> fp16)
            xseg = np.zeros((128, FD), dtype=np.float16)
            for c in range(C):
                sl = slice(c * SC, (c + 1) * SC)
                xseg[:, sl] = g[:, c, sl].astype(np.float16)
            wx = np.ascontiguousarray(np.concatenate([wgrid, xseg], axis=1))
            wsums.append(float(wgrid.astype(np.float64).sum()))
            in_maps.append({"x": xg, "wx": wx})
        round_maps.append(in_maps)
        round_wsums.append(wsums)
    return round_maps, size, SC, round_wsums


def _combine_sum(res_list, wsums) -> float:
    S = 0.0
    for n in range(N_IMG):
        o = np.asarray(res_list[n]["out"], dtype=np.float64)
        S1 = FLOG_S * o[0, 0] - FLOG_B * wsums[n]
        S += o[0, 1] - S1                            # S2 - S1
    return S


def _combine(res_list, size, wsums) -> np.ndarray:
    loss = np.float32(-(_combine_sum(res_list, wsums) / size))
    return np.asarray(loss, dtype=np.float32)


def run_device(in_maps, SC=SC_DEFAULT, trace=False, **kwargs):
    from concourse.bass_utils import run_bass_kernel_spmd

    nc = _get_nc(SC)
    return run_bass_kernel_spmd(
        nc, in_maps, core_ids=list(range(N_IMG)), trace=trace, **kwargs
    )


def kernel(predict, target, confidence, accuracies, n_bin) -> np.ndarray:
    round_maps, size, SC, round_wsums = _prepare(
        predict, target, confidence, accuracies, n_bin
    )
    S = 0.0
    for in_maps, wsums in zip(round_maps, round_wsums):
        res = run_device(in_maps, SC=SC)
        S += _combine_sum(res.results, wsums)
    return np.asarray(np.float32(-(S / size)), dtype=np.float32)
